# revision 63
# baseline (speedup 1.0000x reference)
"""BiDiTreeLSTM forest kernel for 8 Trainium2 NeuronCores.

Strategy (data-parallel over trees, per the sharding hint):
  - 256 complete binary trees (depth 8, 511 nodes); 32 trees per core.
  - Everything on-device is FEATURE-MAJOR: [128 features on partitions,
    nodes on the free axis].  H == X == 128 exactly fills the partitions.
  - Host pre-permutes each core's nodes into a level-grouped "chunk-local
    split" order: level blocks stored t=8..0; within a level, the children
    of the parents in 512-column chunk j form two adjacent 512-column
    chunks 2j (left) and 2j+1 (right).  Every child/parent gather in both
    propagation passes is then a contiguous column slice, dependencies
    between levels are chunk-local, and tree id == column mod 32 at every
    level (512 % 32 == 0).
  - Bottom-up then top-down level-synchronous ChildSum TreeLSTM per core,
    matmuls in bf16 (fp32 PSUM accumulate), elementwise in bf16 on the
    vector engine (2x mode), gates on the scalar engine.
  - The u-gate rows of W/U are pre-scaled by 2 on the host so that
    tanh(u) = 2*sigmoid(2u) - 1 and i,o,u all go through ONE sigmoid
    activation per chunk (the scalar engine is the bottleneck; this cuts
    its instruction count).  Cell states are stored HALVED (c_hat = c/2):
    c_hat = sig(i)*(sig(2u)-0.5) + sum f*c_hat_child, one fused
    scalar_tensor_tensor plus one add on the vector engine, and
    tanh(c) = tanh(2*c_hat) falls out of the activation input scale.
  - tanh(c)/h of chunk k are deferred into chunk k+2 ("phase2") and the
    tanh runs over chunk PAIRS (1024 cols) to halve instruction overhead;
    top-down forget gates are computed for parent-chunk pairs, and the
    leaf-mean tree sums use fp32 halving adds on the vector engine.
  - Single-chunk levels (the latency-bound BU tail / TD head) shorten the
    level-to-level chain: U matmuls run off a vector htild that rides
    behind the h multiply, X-dependent TD matmuls are emitted before the
    parent-dependent ones, and narrow forget-gate matmuls cover both
    children in one instruction.
  - h0/c0 are zeros and b_iou_* are zeros by problem spec; the kernel
    exploits that (they do not affect the output otherwise).
Output per core: [128, 64] fp32 = [root h_bu | leaf-mean h_td] feature-major;
host reassembles the [256, 256] result.
"""

import os
import sys

sys.path.insert(0, "/opt/trn_rl_repo")

import ml_dtypes
import numpy as np

import concourse.bass as bass
import concourse.mybir as mybir
import concourse.tile as tile

B = 256
DEPTH = 8
M = 511
H = 128
NCORES = 8
TPC = B // NCORES            # trees per core = 32
NC_NODES = TPC * M           # 16352
CH = 512                     # moving-dim chunk (one fp32 PSUM bank)

NT = {t: TPC * (1 << t) for t in range(DEPTH + 1)}      # cols per level
OFF = {}
_o = 0
for _t in range(DEPTH, -1, -1):                           # level 8 first
    OFF[_t] = _o
    _o += NT[_t]
assert _o == NC_NODES

F32 = mybir.dt.float32
BF16 = mybir.dt.bfloat16
ActF = mybir.ActivationFunctionType
Alu = mybir.AluOpType

_NC_CACHE = {}


def _split_multi_waits(nc):
    """This container's walrus accepts at most ONE sync wait per
    instruction; Tile attaches several.  Insert single-wait NoOps."""
    n = 0
    for fn in nc.m.functions:
        for bb in fn.blocks:
            insts = bb.instructions
            new = []
            for inst in insts:
                si = inst.sync_info
                if si is not None and si.on_wait and len(si.on_wait) > 1:
                    waits = list(si.on_wait)
                    for j, w in enumerate(waits[:-1]):
                        new.append(mybir.InstNoOp(
                            name=f"{inst.name}_w{j}",
                            sync_info=mybir.SyncInfo(on_wait=[w], on_update=[]),
                            bass_nofuse=True,
                            engine=inst.engine,
                        ))
                        n += 1
                    si.on_wait = [waits[-1]]
                new.append(inst)
            if len(new) != len(insts):
                bb.instructions[:] = new
    return n


def _build_nc():
    nc = bass.Bass("TRN2")

    XT = nc.dram_tensor("XT", [H, NC_NODES], BF16, kind="ExternalInput")
    # all weight matrices packed: [WBU | UBU | UFBU | WTDX | WTDH | UTD | UFTD]
    WPACK = nc.dram_tensor("WPACK", [H, 2176], BF16, kind="ExternalInput")
    BPACK = nc.dram_tensor("BPACK", [H, 2], F32, kind="ExternalInput")
    OUT = nc.dram_tensor("OUT", [H, 64], F32, kind="ExternalOutput")

    with tile.TileContext(nc) as tc:
        with tc.tile_pool(name="persist", bufs=1) as P, \
             tc.tile_pool(name="ws", bufs=4) as W, \
             tc.tile_pool(name="wdeep", bufs=6) as WD, \
             tc.tile_pool(name="psiou", bufs=2, space="PSUM") as PI, \
             tc.tile_pool(name="psf", bufs=1, space="PSUM") as PF:

            # ---- persistent SBUF ----
            xt = P.tile([H, NC_NODES], BF16)
            wpk = P.tile([H, 2176], BF16)
            bpk = P.tile([H, 2], F32)
            # xt streamed on the SWDGE (gpsimd) queue so its issue cost
            # overlaps the sync-queue weight loads; graded chunks so level-8
            # compute starts as soon as the first columns land
            # first leaf chunk + leaf-level W on the fast sync queue so the
            # first matmul starts as early as possible; everything else
            # streams behind on both queues
            nc.sync.dma_start(out=xt[:, 0:256], in_=XT[:, 0:256])
            nc.sync.dma_start(out=wpk[:, 0:384], in_=WPACK[:, 0:384])
            nc.gpsimd.dma_start(out=xt[:, 256:1536], in_=XT[:, 256:1536])
            nc.sync.dma_start(out=wpk[:, 384:2176], in_=WPACK[:, 384:2176])
            nc.sync.dma_start(out=bpk[:], in_=BPACK[:])
            for a, b in ((1536, 4096), (4096, 10240), (10240, NC_NODES)):
                nc.gpsimd.dma_start(out=xt[:, a:b], in_=XT[:, a:b])
            wbu = wpk[:, 0:384]
            ubu = wpk[:, 384:768]
            ufbu = wpk[:, 768:896]
            wtdx = wpk[:, 896:1280]
            wtdh = wpk[:, 1280:1664]
            utd = wpk[:, 1664:2048]
            uftd = wpk[:, 2048:2176]
            bfbu = bpk[:, 0:1]
            bftd = bpk[:, 1:2]

            hbu = P.tile([H, NC_NODES], BF16)     # all bottom-up h
            ca = P.tile([H, 8192], BF16)          # c ping (even levels)
            cb = P.tile([H, 4096], BF16)          # c pong (odd levels)
            ha = P.tile([H, 2048], BF16)          # td h ping (even, t<8)
            hb = P.tile([H, 4096], BF16)          # td h pong (odd)
            slots = P.tile([H, 288], F32)         # leaf-mean partials
            outsb = P.tile([H, 64], F32)

            def cbuf(t):
                return ca if t % 2 == 0 else cb

            def hbuf(t):
                return ha if t % 2 == 0 else hb

            # ---------------- bottom-up ----------------
            # phase2 (tanh(c), h) of chunk k is emitted during chunk k+2;
            # with the chunk-local layout later chunks never need the two
            # immediately-preceding chunks' h, so deferral is safe even
            # across level boundaries (small levels flush at level end).
            def bu_phase2_one(st):
                t, c0, w, G, sio = st
                cdst = cbuf(t)[:, c0: c0 + w]
                tch = W.tile([H, CH], BF16, tag="tc")
                nc.scalar.activation(out=tch[:, :w], in_=cdst,
                                     func=ActF.Tanh, scale=2.0)
                nc.vector.tensor_mul(hbu[:, OFF[t] + c0: OFF[t] + c0 + w],
                                     sio[:, G:G + w], tch[:, :w])
                if t == 0:
                    # fp32 root output
                    nc.vector.tensor_mul(outsb[:, 0:32],
                                         sio[:, G:G + w], tch[:, :w])

            def bu_phase2_pair(s1, s2):
                t, c0, w, G, sio1 = s1
                _, c0b, wb, Gb, sio2 = s2
                tch = WD.tile([H, 2 * CH], BF16, tag="tc2")
                nc.scalar.activation(out=tch[:, :w + wb],
                                     in_=cbuf(t)[:, c0: c0 + w + wb],
                                     func=ActF.Tanh, scale=2.0)
                nc.vector.tensor_mul(hbu[:, OFF[t] + c0: OFF[t] + c0 + w],
                                     sio1[:, G:G + w], tch[:, :w])
                nc.vector.tensor_mul(hbu[:, OFF[t] + c0b: OFF[t] + c0b + wb],
                                     sio2[:, Gb:Gb + wb], tch[:, w:w + wb])

            def bu_flush(pend):
                while pend:
                    if (len(pend) >= 2 and pend[0][0] == pend[1][0]
                            and pend[1][1] == pend[0][1] + pend[0][2]):
                        bu_phase2_pair(pend[0], pend[1])
                        del pend[:2]
                    else:
                        bu_phase2_one(pend[0])
                        del pend[:1]

            pend = []
            for t in range(DEPTH, -1, -1):
                n = NT[t]
                xoff = OFF[t]
                cw = cbuf(t)
                for c0 in range(0, n, CH):
                    w = min(CH, n - c0)
                    # gate k lives at col k*G; start=True exactly on the
                    # first matmul touching each PSUM bank (start clears
                    # bank-wide has_written bits)
                    G = w if w < CH else CH
                    iou = PI.tile([H, 3 * CH], F32, tag="iou")
                    for k in range(3):
                        nc.tensor.matmul(
                            iou[:, k * G: k * G + w],
                            wbu[:, k * H:(k + 1) * H],
                            xt[:, xoff + c0: xoff + c0 + w],
                            start=(k * G) % 512 == 0, stop=(t == DEPTH))
                    if t < DEPTH:
                        # children of parents [c0:c0+w] are the adjacent
                        # chunks [2c0 : 2c0+w] and [2c0+w : 2c0+2w]
                        choff = OFF[t + 1]
                        hl = hbu[:, choff + 2 * c0: choff + 2 * c0 + w]
                        hr = hbu[:, choff + 2 * c0 + w: choff + 2 * c0 + 2 * w]
                        cc = cbuf(t + 1)
                        psf = PF.tile([H, 2 * CH], F32, tag="pf")
                        if 2 * w <= CH:
                            # both children contiguous in one PSUM bank:
                            # single forget-gate matmul on the chain
                            nc.tensor.matmul(
                                psf[:, 0:2 * w], ufbu,
                                hbu[:, choff + 2 * c0: choff + 2 * c0 + 2 * w],
                                start=True, stop=True)
                        else:
                            nc.tensor.matmul(psf[:, 0:w], ufbu, hl,
                                             start=True, stop=True)
                            nc.tensor.matmul(psf[:, G:G + w], ufbu, hr,
                                             start=G % 512 == 0, stop=True)
                        # htild on vector rides right behind the h-mul that
                        # produced hl/hr
                        htild = W.tile([H, CH], BF16, tag="htild")
                        nc.vector.tensor_add(htild[:, :w], hl, hr)
                        for k in range(3):
                            nc.tensor.matmul(
                                iou[:, k * G: k * G + w],
                                ubu[:, k * H:(k + 1) * H],
                                htild[:, :w],
                                start=False, stop=True)
                        f = WD.tile([H, 2 * CH], BF16, tag="f")
                        nc.scalar.activation(out=f[:, :G + w],
                                             in_=psf[:, :G + w],
                                             func=ActF.Sigmoid, bias=bfbu)
                        # fc over both (contiguous) children, then fold
                        fc2 = W.tile([H, 2 * CH], BF16, tag="fc2")
                        nc.vector.tensor_mul(fc2[:, :2 * w], f[:, :2 * w],
                                             cc[:, 2 * c0: 2 * c0 + 2 * w])
                        cred = W.tile([H, CH], BF16, tag="cred")
                        nc.vector.tensor_add(cred[:, :w], fc2[:, :w],
                                             fc2[:, w:2 * w])
                    sio = WD.tile([H, 3 * CH], BF16, tag="sio")
                    cdst = cw[:, c0: c0 + w]
                    # one sigmoid for i,o,2u; c_hat = sig(i)*(s-0.5)
                    # [+ cred] with s = sig(2u)
                    nc.scalar.activation(out=sio[:, :3 * G],
                                         in_=iou[:, 0:3 * G],
                                         func=ActF.Sigmoid)
                    if t < DEPTH:
                        t1 = W.tile([H, CH], BF16, tag="t1")
                        nc.vector.scalar_tensor_tensor(
                            t1[:, :w], sio[:, 2 * G:2 * G + w], 0.5,
                            sio[:, 0:w], Alu.subtract, Alu.mult)
                        nc.vector.tensor_add(cdst, t1[:, :w],
                                             cred[:, :w])
                    else:
                        nc.vector.scalar_tensor_tensor(
                            cdst, sio[:, 2 * G:2 * G + w], 0.5,
                            sio[:, 0:w], Alu.subtract, Alu.mult)
                    if len(pend) >= 2:
                        bu_flush(pend)
                    pend.append((t, c0, w, G, sio))
                if n <= 2 * CH:
                    # small level: the next level's first chunk reads this
                    # level's last chunks -- deferral would skip the RAW dep
                    bu_flush(pend)

            # ---------------- top-down ----------------
            def td_phase2_one(st):
                t, coff, w, G, sio = st
                cdst = cbuf(t)[:, coff: coff + w]
                tch = W.tile([H, CH], BF16, tag="tc")
                nc.scalar.activation(out=tch[:, :w], in_=cdst,
                                     func=ActF.Tanh, scale=2.0)
                if t < DEPTH:
                    nc.vector.tensor_mul(hbuf(t)[:, coff: coff + w],
                                         sio[:, G:G + w], tch[:, :w])
                else:
                    # lone leaf chunk (only the final two, flushed singly
                    # to shorten the drain tail); odd chunks get their own
                    # slot group past the pair groups
                    assert w == CH
                    hn = W.tile([H, CH], BF16, tag="hn")
                    nc.vector.tensor_mul(hn[:, :w],
                                         sio[:, G:G + w], tch[:, :w])
                    ci = coff // CH
                    qoff = ci * 16 + (16 if ci % 2 == 1 else 0)
                    r2 = W.tile([H, CH], F32, tag="r2")
                    nc.vector.tensor_add(r2[:, 0:256], hn[:, 0:256],
                                         hn[:, 256:512])
                    nc.vector.tensor_add(r2[:, 256:384], r2[:, 0:128],
                                         r2[:, 128:256])
                    nc.vector.tensor_add(r2[:, 384:448], r2[:, 256:320],
                                         r2[:, 320:384])
                    nc.vector.tensor_add(slots[:, qoff:qoff + 32],
                                         r2[:, 384:416], r2[:, 416:448])

            def td_phase2_pair(s1, s2):
                t, coff, w, G, sio1 = s1
                _, coffb, wb, Gb, sio2 = s2
                tch = WD.tile([H, 2 * CH], BF16, tag="tc2")
                nc.scalar.activation(out=tch[:, :w + wb],
                                     in_=cbuf(t)[:, coff: coff + w + wb],
                                     func=ActF.Tanh, scale=2.0)
                if t < DEPTH:
                    nc.vector.tensor_mul(hbuf(t)[:, coff: coff + w],
                                         sio1[:, G:G + w], tch[:, :w])
                    nc.vector.tensor_mul(hbuf(t)[:, coffb: coffb + wb],
                                         sio2[:, Gb:Gb + wb], tch[:, w:w + wb])
                else:
                    assert w == CH and wb == CH
                    hn = W.tile([H, 2 * CH], BF16, tag="hn2")
                    nc.vector.tensor_mul(hn[:, :w],
                                         sio1[:, G:G + w], tch[:, :w])
                    nc.vector.tensor_mul(hn[:, w:w + wb],
                                         sio2[:, Gb:Gb + wb], tch[:, w:w + wb])
                    ci = coff // CH
                    # tree ids align mod 32 across both chunks: per-tree
                    # sums via fp32 halving adds (cheaper than the strided
                    # 32-group reduce on this engine)
                    r1 = W.tile([H, CH], F32, tag="r1")
                    nc.vector.tensor_add(r1[:, 0:512], hn[:, 0:512],
                                         hn[:, 512:1024])
                    r2 = W.tile([H, CH], F32, tag="r2")
                    nc.vector.tensor_add(r2[:, 0:256], r1[:, 0:256],
                                         r1[:, 256:512])
                    nc.vector.tensor_add(r2[:, 256:384], r2[:, 0:128],
                                         r2[:, 128:256])
                    nc.vector.tensor_add(r2[:, 384:448], r2[:, 256:320],
                                         r2[:, 320:384])
                    nc.vector.tensor_add(slots[:, ci * 16:ci * 16 + 32],
                                         r2[:, 384:416], r2[:, 416:448])

            def td_flush(pend):
                while pend:
                    if (len(pend) >= 2 and pend[0][0] == pend[1][0]
                            and pend[1][1] == pend[0][1] + pend[0][2]):
                        td_phase2_pair(pend[0], pend[1])
                        del pend[:2]
                    else:
                        td_phase2_one(pend[0])
                        del pend[:1]

            pend = []
            for t in range(0, DEPTH + 1):
                n = NT[t]
                xoff = OFF[t]
                cw = cbuf(t)
                if t == 0:
                    w = n  # 32
                    G = w
                    iou = PI.tile([H, 3 * CH], F32, tag="iou")
                    for k in range(3):
                        nc.tensor.matmul(iou[:, k * G: k * G + w],
                                         wtdx[:, k * H:(k + 1) * H],
                                         xt[:, xoff: xoff + w],
                                         start=(k == 0), stop=False)
                        nc.tensor.matmul(iou[:, k * G: k * G + w],
                                         wtdh[:, k * H:(k + 1) * H],
                                         hbu[:, xoff: xoff + w],
                                         start=False, stop=True)
                    sio = WD.tile([H, 3 * CH], BF16, tag="sio")
                    nc.scalar.activation(out=sio[:, :3 * G],
                                         in_=iou[:, 0:3 * G],
                                         func=ActF.Sigmoid)
                    nc.vector.scalar_tensor_tensor(
                        cw[:, 0:w], sio[:, 2 * G:2 * G + w], 0.5,
                        sio[:, 0:w], Alu.subtract, Alu.mult)
                    td_phase2_one((0, 0, w, G, sio))
                    continue
                half = n // 2
                hp = hbuf(t - 1)
                cp = cbuf(t - 1)
                if half <= CH:
                    # chain level (one parent chunk): X-dependent matmuls
                    # first so the PE chews them while the parent h chain
                    # resolves; psf/utd land right when hp is ready
                    w = half
                    G = w if w < CH else CH
                    ious = []
                    for side in range(2):
                        coff = side * w
                        iou = PI.tile([H, 3 * CH], F32, tag="iou")
                        for k in range(3):
                            nc.tensor.matmul(
                                iou[:, k * G: k * G + w],
                                wtdx[:, k * H:(k + 1) * H],
                                xt[:, xoff + coff: xoff + coff + w],
                                start=(k * G) % 512 == 0, stop=False)
                        for k in range(3):
                            nc.tensor.matmul(
                                iou[:, k * G: k * G + w],
                                wtdh[:, k * H:(k + 1) * H],
                                hbu[:, xoff + coff: xoff + coff + w],
                                start=False, stop=False)
                        ious.append(iou)
                    psf = PF.tile([H, 2 * CH], F32, tag="pf")
                    nc.tensor.matmul(psf[:, 0:w], uftd, hp[:, 0:w],
                                     start=True, stop=True)
                    for side in range(2):
                        for k in range(3):
                            nc.tensor.matmul(
                                ious[side][:, k * G: k * G + w],
                                utd[:, k * H:(k + 1) * H],
                                hp[:, 0:w], start=False, stop=True)
                    f = WD.tile([H, 2 * CH], BF16, tag="f")
                    nc.scalar.activation(out=f[:, :w], in_=psf[:, :w],
                                         func=ActF.Sigmoid, bias=bftd)
                    sios = []
                    for side in range(2):
                        sio = WD.tile([H, 3 * CH], BF16, tag="sio")
                        nc.scalar.activation(out=sio[:, :3 * G],
                                             in_=ious[side][:, 0:3 * G],
                                             func=ActF.Sigmoid)
                        sios.append(sio)
                    fc = W.tile([H, 2 * CH], BF16, tag="fcl")
                    nc.vector.tensor_mul(fc[:, :w], f[:, :w], cp[:, 0:w])
                    for side in range(2):
                        coff = side * w
                        t1 = W.tile([H, CH], BF16, tag="t1")
                        nc.vector.scalar_tensor_tensor(
                            t1[:, :w], sios[side][:, 2 * G:2 * G + w], 0.5,
                            sios[side][:, 0:w], Alu.subtract, Alu.mult)
                        nc.vector.tensor_add(cw[:, coff: coff + w],
                                             t1[:, :w], fc[:, :w])
                        pend.append((t, coff, w, G, sios[side]))
                    td_flush(pend)
                    continue
                # parent chunks processed in pairs sharing one psf tile and
                # one forget-gate sigmoid / fc multiply
                p0 = 0
                while p0 < half:
                    npair = 2 if p0 + CH < half else 1
                    wps = [min(CH, half - (p0 + j * CH)) for j in range(npair)]
                    psf = PF.tile([H, 2 * CH], F32, tag="pf")
                    for j in range(npair):
                        nc.tensor.matmul(psf[:, j * CH: j * CH + wps[j]],
                                         uftd,
                                         hp[:, p0 + j * CH: p0 + j * CH + wps[j]],
                                         start=True, stop=True)
                    fw = (npair - 1) * CH + wps[-1]
                    f = WD.tile([H, 2 * CH], BF16, tag="f")
                    nc.scalar.activation(out=f[:, :fw], in_=psf[:, :fw],
                                         func=ActF.Sigmoid, bias=bftd)
                    fc = W.tile([H, 2 * CH], BF16, tag="fcl")
                    nc.vector.tensor_mul(fc[:, :fw], f[:, :fw],
                                         cp[:, p0: p0 + fw])
                    for j in range(npair):
                        pj = p0 + j * CH
                        w = wps[j]
                        G = w if w < CH else CH
                        for side in range(2):
                            coff = 2 * pj + side * w
                            iou = PI.tile([H, 3 * CH], F32, tag="iou")
                            for k in range(3):
                                nc.tensor.matmul(
                                    iou[:, k * G: k * G + w],
                                    wtdx[:, k * H:(k + 1) * H],
                                    xt[:, xoff + coff: xoff + coff + w],
                                    start=(k * G) % 512 == 0,
                                    stop=False)
                            for k in range(3):
                                nc.tensor.matmul(
                                    iou[:, k * G: k * G + w],
                                    wtdh[:, k * H:(k + 1) * H],
                                    hbu[:, xoff + coff: xoff + coff + w],
                                    start=False, stop=False)
                            for k in range(3):
                                nc.tensor.matmul(
                                    iou[:, k * G: k * G + w],
                                    utd[:, k * H:(k + 1) * H],
                                    hp[:, pj: pj + w],
                                    start=False, stop=True)
                            sio = WD.tile([H, 3 * CH], BF16, tag="sio")
                            fcs = fc[:, j * CH: j * CH + w]
                            nc.scalar.activation(out=sio[:, :3 * G],
                                                 in_=iou[:, 0:3 * G],
                                                 func=ActF.Sigmoid)
                            t1 = W.tile([H, CH], BF16, tag="t1")
                            nc.vector.scalar_tensor_tensor(
                                t1[:, :w], sio[:, 2 * G:2 * G + w], 0.5,
                                sio[:, 0:w], Alu.subtract, Alu.mult)
                            nc.vector.tensor_add(
                                cw[:, coff: coff + w], t1[:, :w], fcs)
                            if len(pend) >= 2:
                                td_flush(pend)
                            pend.append((t, coff, w, G, sio))
                    p0 += npair * CH
                if n <= 2 * CH:
                    td_flush(pend)
            # partial leaf-mean combine over the 7 finished pair groups
            # BEFORE flushing the final two chunks -- it overlaps their
            # gate math, leaving only two 32-col accumulates on the tail
            nc.vector.reduce_sum(
                out=outsb[:, 32:64],
                in_=slots[:, 0:224].rearrange("p (k t) -> p t k", t=32),
                axis=mybir.AxisListType.X)
            # flush the final leaf chunks SINGLY: the second-to-last
            # chunk's reduction then overlaps the last chunk's gate math,
            # shortening the serial drain tail
            while pend:
                td_phase2_one(pend.pop(0))
            nc.vector.tensor_add(outsb[:, 32:64], outsb[:, 32:64],
                                 slots[:, 224:256])
            nc.vector.tensor_add(outsb[:, 32:64], outsb[:, 32:64],
                                 slots[:, 256:288])
            nc.vector.tensor_scalar_mul(outsb[:, 32:64], outsb[:, 32:64],
                                        1.0 / 256.0)
            nc.sync.dma_start(out=OUT[:], in_=outsb[:])

    _split_multi_waits(nc)
    return nc


def _perm():
    """Per-core node permutation: level-grouped chunk-local-split order.
    Entry = row index into the core's [16352, 128] X slab."""
    trees = np.arange(TPC, dtype=np.int64)
    heap = [np.zeros(TPC, dtype=np.int64)]
    tree = [trees.copy()]
    for t in range(1, DEPTH + 1):
        ph, pt = heap[t - 1], tree[t - 1]
        nh, ntr = [], []
        for j in range(0, len(ph), CH):
            bh = ph[j:j + CH]
            bt = pt[j:j + CH]
            nh.append(2 * bh + 1)
            nh.append(2 * bh + 2)
            ntr.append(bt)
            ntr.append(bt)
        heap.append(np.concatenate(nh))
        tree.append(np.concatenate(ntr))
    parts = [tree[t] * M + heap[t] for t in range(DEPTH, -1, -1)]
    return np.concatenate(parts)


def kernel(**inputs):
    from concourse.bass_utils import run_bass_kernel_spmd

    X = np.asarray(inputs["X"], dtype=np.float32)
    W_iou_bu = np.asarray(inputs["W_iou_bu"], dtype=np.float32)
    U_iou_bu = np.asarray(inputs["U_iou_bu"], dtype=np.float32)
    Uf_bu = np.asarray(inputs["Uf_bu"], dtype=np.float32)
    bf_bu = np.asarray(inputs["bf_bu"], dtype=np.float32)
    W_iou_td = np.asarray(inputs["W_iou_td"], dtype=np.float32)
    U_iou_td = np.asarray(inputs["U_iou_td"], dtype=np.float32)
    Uf_td = np.asarray(inputs["Uf_td"], dtype=np.float32)
    bf_td = np.asarray(inputs["bf_td"], dtype=np.float32)

    bf16 = ml_dtypes.bfloat16
    wpack = np.concatenate([
        W_iou_bu.T, U_iou_bu.T, Uf_bu.T,
        W_iou_td[:, :H].T, W_iou_td[:, H:].T, U_iou_td.T, Uf_td.T,
    ], axis=1).copy()
    # u-gate rows pre-scaled by 2: tanh(u) = 2*sigmoid(2u) - 1 on device
    for base in (0, 384, 896, 1280, 1664):
        wpack[:, base + 2 * H: base + 3 * H] *= 2.0
    bpack = np.stack([bf_bu, bf_td], axis=1)
    shared = {
        "WPACK": np.ascontiguousarray(wpack).astype(bf16),
        "BPACK": np.ascontiguousarray(bpack, dtype=np.float32),
    }
    perm = _perm()
    in_maps = []
    for c in range(NCORES):
        slab = X[c * NC_NODES:(c + 1) * NC_NODES]
        xtc = np.ascontiguousarray(slab[perm].T.astype(bf16))
        m = dict(shared)
        m["XT"] = xtc
        in_maps.append(m)

    if "nc" not in _NC_CACHE:
        _NC_CACHE["nc"] = _build_nc()
    nc = _NC_CACHE["nc"]

    trace = bool(os.environ.get("BIDI_TRACE"))
    if trace:
        sys.path.insert(0, "/root/problem/work")
        try:
            import ntff_hook
            ntff_hook.install()
        except Exception:
            trace = False
    res = run_bass_kernel_spmd(nc, in_maps, core_ids=list(range(NCORES)),
                               trace=trace)
    global LAST_EXEC_NS, LAST_TRACE
    LAST_EXEC_NS = res.exec_time_ns
    LAST_TRACE = res.instructions_and_trace

    out = np.empty((B, 2 * H), dtype=np.float32)
    for c in range(NCORES):
        o = res.results[c]["OUT"]          # [128, 64]
        out[c * TPC:(c + 1) * TPC, :H] = o[:, 0:32].T
        out[c * TPC:(c + 1) * TPC, H:] = o[:, 32:64].T
    return out


LAST_EXEC_NS = None
LAST_TRACE = None


# revision 64
# speedup vs baseline: 1.0027x; 1.0027x over previous
"""BiDiTreeLSTM forest kernel for 8 Trainium2 NeuronCores.

Strategy (data-parallel over trees, per the sharding hint):
  - 256 complete binary trees (depth 8, 511 nodes); 32 trees per core.
  - Everything on-device is FEATURE-MAJOR: [128 features on partitions,
    nodes on the free axis].  H == X == 128 exactly fills the partitions.
  - Host pre-permutes each core's nodes into a level-grouped "chunk-local
    split" order: level blocks stored t=8..0; within a level, the children
    of the parents in 512-column chunk j form two adjacent 512-column
    chunks 2j (left) and 2j+1 (right).  Every child/parent gather in both
    propagation passes is then a contiguous column slice, dependencies
    between levels are chunk-local, and tree id == column mod 32 at every
    level (512 % 32 == 0).
  - Bottom-up then top-down level-synchronous ChildSum TreeLSTM per core,
    matmuls in bf16 (fp32 PSUM accumulate), elementwise in bf16 on the
    vector engine (2x mode), gates on the scalar engine.
  - The u-gate rows of W/U are pre-scaled by 2 on the host so that
    tanh(u) = 2*sigmoid(2u) - 1 and i,o,u all go through ONE sigmoid
    activation per chunk (the scalar engine is the bottleneck; this cuts
    its instruction count).  Cell states are stored HALVED (c_hat = c/2):
    c_hat = sig(i)*(sig(2u)-0.5) + sum f*c_hat_child, one fused
    scalar_tensor_tensor plus one add on the vector engine, and
    tanh(c) = tanh(2*c_hat) falls out of the activation input scale.
  - tanh(c)/h of chunk k are deferred into chunk k+2 ("phase2") and the
    tanh runs over chunk PAIRS (1024 cols) to halve instruction overhead;
    top-down forget gates are computed for parent-chunk pairs, and the
    leaf-mean tree sums use fp32 halving adds on the vector engine.
  - Single-chunk levels (the latency-bound BU tail / TD head) shorten the
    level-to-level chain: U matmuls run off a vector htild that rides
    behind the h multiply, X-dependent TD matmuls are emitted before the
    parent-dependent ones, and narrow forget-gate matmuls cover both
    children in one instruction.
  - h0/c0 are zeros and b_iou_* are zeros by problem spec; the kernel
    exploits that (they do not affect the output otherwise).
Output per core: [128, 64] fp32 = [root h_bu | leaf-mean h_td] feature-major;
host reassembles the [256, 256] result.
"""

import os
import sys

sys.path.insert(0, "/opt/trn_rl_repo")

import ml_dtypes
import numpy as np

import concourse.bass as bass
import concourse.mybir as mybir
import concourse.tile as tile

B = 256
DEPTH = 8
M = 511
H = 128
NCORES = 8
TPC = B // NCORES            # trees per core = 32
NC_NODES = TPC * M           # 16352
CH = 512                     # moving-dim chunk (one fp32 PSUM bank)

NT = {t: TPC * (1 << t) for t in range(DEPTH + 1)}      # cols per level
OFF = {}
_o = 0
for _t in range(DEPTH, -1, -1):                           # level 8 first
    OFF[_t] = _o
    _o += NT[_t]
assert _o == NC_NODES

F32 = mybir.dt.float32
BF16 = mybir.dt.bfloat16
ActF = mybir.ActivationFunctionType
Alu = mybir.AluOpType

_NC_CACHE = {}


def _split_multi_waits(nc):
    """This container's walrus accepts at most ONE sync wait per
    instruction; Tile attaches several.  Insert single-wait NoOps."""
    n = 0
    for fn in nc.m.functions:
        for bb in fn.blocks:
            insts = bb.instructions
            new = []
            for inst in insts:
                si = inst.sync_info
                if si is not None and si.on_wait and len(si.on_wait) > 1:
                    waits = list(si.on_wait)
                    for j, w in enumerate(waits[:-1]):
                        new.append(mybir.InstNoOp(
                            name=f"{inst.name}_w{j}",
                            sync_info=mybir.SyncInfo(on_wait=[w], on_update=[]),
                            bass_nofuse=True,
                            engine=inst.engine,
                        ))
                        n += 1
                    si.on_wait = [waits[-1]]
                new.append(inst)
            if len(new) != len(insts):
                bb.instructions[:] = new
    return n


def _build_nc():
    nc = bass.Bass("TRN2")

    XT = nc.dram_tensor("XT", [H, NC_NODES], BF16, kind="ExternalInput")
    # all weight matrices packed: [WBU | UBU | UFBU | WTDX | WTDH | UTD | UFTD]
    WPACK = nc.dram_tensor("WPACK", [H, 2176], BF16, kind="ExternalInput")
    BPACK = nc.dram_tensor("BPACK", [H, 2], F32, kind="ExternalInput")
    OUT = nc.dram_tensor("OUT", [H, 64], F32, kind="ExternalOutput")

    with tile.TileContext(nc) as tc:
        with tc.tile_pool(name="persist", bufs=1) as P, \
             tc.tile_pool(name="ws", bufs=4) as W, \
             tc.tile_pool(name="wdeep", bufs=6) as WD, \
             tc.tile_pool(name="psiou", bufs=2, space="PSUM") as PI, \
             tc.tile_pool(name="psf", bufs=1, space="PSUM") as PF:

            # ---- persistent SBUF ----
            xt = P.tile([H, NC_NODES], BF16)
            wpk = P.tile([H, 2176], BF16)
            bpk = P.tile([H, 2], F32)
            # xt streamed on the SWDGE (gpsimd) queue so its issue cost
            # overlaps the sync-queue weight loads; graded chunks so level-8
            # compute starts as soon as the first columns land
            # first leaf chunk + leaf-level W on the fast sync queue so the
            # first matmul starts as early as possible; everything else
            # streams behind on both queues
            nc.sync.dma_start(out=xt[:, 0:256], in_=XT[:, 0:256])
            nc.sync.dma_start(out=wpk[:, 0:384], in_=WPACK[:, 0:384])
            nc.gpsimd.dma_start(out=xt[:, 256:1536], in_=XT[:, 256:1536])
            nc.sync.dma_start(out=wpk[:, 384:2176], in_=WPACK[:, 384:2176])
            nc.sync.dma_start(out=bpk[:], in_=BPACK[:])
            for a, b in ((1536, 4096), (4096, 10240), (10240, NC_NODES)):
                nc.gpsimd.dma_start(out=xt[:, a:b], in_=XT[:, a:b])
            wbu = wpk[:, 0:384]
            ubu = wpk[:, 384:768]
            ufbu = wpk[:, 768:896]
            wtdx = wpk[:, 896:1280]
            wtdh = wpk[:, 1280:1664]
            utd = wpk[:, 1664:2048]
            uftd = wpk[:, 2048:2176]
            bfbu = bpk[:, 0:1]
            bftd = bpk[:, 1:2]

            hbu = P.tile([H, NC_NODES], BF16)     # all bottom-up h
            ca = P.tile([H, 8192], BF16)          # c ping (even levels)
            cb = P.tile([H, 4096], BF16)          # c pong (odd levels)
            ha = P.tile([H, 2048], BF16)          # td h ping (even, t<8)
            hb = P.tile([H, 4096], BF16)          # td h pong (odd)
            slots = P.tile([H, 288], F32)         # leaf-mean partials
            outsb = P.tile([H, 64], F32)

            def cbuf(t):
                return ca if t % 2 == 0 else cb

            def hbuf(t):
                return ha if t % 2 == 0 else hb

            # ---------------- bottom-up ----------------
            # phase2 (tanh(c), h) of chunk k is emitted during chunk k+2;
            # with the chunk-local layout later chunks never need the two
            # immediately-preceding chunks' h, so deferral is safe even
            # across level boundaries (small levels flush at level end).
            def bu_phase2_one(st):
                t, c0, w, G, sio = st
                cdst = cbuf(t)[:, c0: c0 + w]
                tch = W.tile([H, CH], BF16, tag="tc")
                nc.scalar.activation(out=tch[:, :w], in_=cdst,
                                     func=ActF.Tanh, scale=2.0)
                nc.vector.tensor_mul(hbu[:, OFF[t] + c0: OFF[t] + c0 + w],
                                     sio[:, G:G + w], tch[:, :w])
                if t == 0:
                    # fp32 root output
                    nc.vector.tensor_mul(outsb[:, 0:32],
                                         sio[:, G:G + w], tch[:, :w])

            def bu_phase2_pair(s1, s2):
                t, c0, w, G, sio1 = s1
                _, c0b, wb, Gb, sio2 = s2
                tch = WD.tile([H, 2 * CH], BF16, tag="tc2")
                nc.scalar.activation(out=tch[:, :w + wb],
                                     in_=cbuf(t)[:, c0: c0 + w + wb],
                                     func=ActF.Tanh, scale=2.0)
                nc.vector.tensor_mul(hbu[:, OFF[t] + c0: OFF[t] + c0 + w],
                                     sio1[:, G:G + w], tch[:, :w])
                nc.vector.tensor_mul(hbu[:, OFF[t] + c0b: OFF[t] + c0b + wb],
                                     sio2[:, Gb:Gb + wb], tch[:, w:w + wb])

            def bu_flush(pend):
                while pend:
                    if (len(pend) >= 2 and pend[0][0] == pend[1][0]
                            and pend[1][1] == pend[0][1] + pend[0][2]):
                        bu_phase2_pair(pend[0], pend[1])
                        del pend[:2]
                    else:
                        bu_phase2_one(pend[0])
                        del pend[:1]

            pend = []
            for t in range(DEPTH, -1, -1):
                n = NT[t]
                xoff = OFF[t]
                cw = cbuf(t)
                for c0 in range(0, n, CH):
                    w = min(CH, n - c0)
                    # gate k lives at col k*G; start=True exactly on the
                    # first matmul touching each PSUM bank (start clears
                    # bank-wide has_written bits)
                    G = w if w < CH else CH
                    iou = PI.tile([H, 3 * CH], F32, tag="iou")
                    for k in range(3):
                        nc.tensor.matmul(
                            iou[:, k * G: k * G + w],
                            wbu[:, k * H:(k + 1) * H],
                            xt[:, xoff + c0: xoff + c0 + w],
                            start=(k * G) % 512 == 0, stop=(t == DEPTH))
                    if t < DEPTH:
                        # children of parents [c0:c0+w] are the adjacent
                        # chunks [2c0 : 2c0+w] and [2c0+w : 2c0+2w]
                        choff = OFF[t + 1]
                        hl = hbu[:, choff + 2 * c0: choff + 2 * c0 + w]
                        hr = hbu[:, choff + 2 * c0 + w: choff + 2 * c0 + 2 * w]
                        cc = cbuf(t + 1)
                        psf = PF.tile([H, 2 * CH], F32, tag="pf")
                        if 2 * w <= CH:
                            # both children contiguous in one PSUM bank:
                            # single forget-gate matmul on the chain
                            nc.tensor.matmul(
                                psf[:, 0:2 * w], ufbu,
                                hbu[:, choff + 2 * c0: choff + 2 * c0 + 2 * w],
                                start=True, stop=True)
                        else:
                            nc.tensor.matmul(psf[:, 0:w], ufbu, hl,
                                             start=True, stop=True)
                            nc.tensor.matmul(psf[:, G:G + w], ufbu, hr,
                                             start=G % 512 == 0, stop=True)
                        # htild on vector rides right behind the h-mul that
                        # produced hl/hr
                        htild = W.tile([H, CH], BF16, tag="htild")
                        nc.vector.tensor_add(htild[:, :w], hl, hr)
                        for k in range(3):
                            nc.tensor.matmul(
                                iou[:, k * G: k * G + w],
                                ubu[:, k * H:(k + 1) * H],
                                htild[:, :w],
                                start=False, stop=True)
                        f = WD.tile([H, 2 * CH], BF16, tag="f")
                        nc.scalar.activation(out=f[:, :G + w],
                                             in_=psf[:, :G + w],
                                             func=ActF.Sigmoid, bias=bfbu)
                        # fc over both (contiguous) children, then fold
                        fc2 = W.tile([H, 2 * CH], BF16, tag="fc2")
                        nc.vector.tensor_mul(fc2[:, :2 * w], f[:, :2 * w],
                                             cc[:, 2 * c0: 2 * c0 + 2 * w])
                        cred = W.tile([H, CH], BF16, tag="cred")
                        nc.vector.tensor_add(cred[:, :w], fc2[:, :w],
                                             fc2[:, w:2 * w])
                    sio = WD.tile([H, 3 * CH], BF16, tag="sio")
                    cdst = cw[:, c0: c0 + w]
                    # one sigmoid for i,o,2u; c_hat = sig(i)*(s-0.5)
                    # [+ cred] with s = sig(2u)
                    nc.scalar.activation(out=sio[:, :3 * G],
                                         in_=iou[:, 0:3 * G],
                                         func=ActF.Sigmoid)
                    if t < DEPTH:
                        t1 = W.tile([H, CH], BF16, tag="t1")
                        nc.vector.scalar_tensor_tensor(
                            t1[:, :w], sio[:, 2 * G:2 * G + w], 0.5,
                            sio[:, 0:w], Alu.subtract, Alu.mult)
                        nc.vector.tensor_add(cdst, t1[:, :w],
                                             cred[:, :w])
                    else:
                        nc.vector.scalar_tensor_tensor(
                            cdst, sio[:, 2 * G:2 * G + w], 0.5,
                            sio[:, 0:w], Alu.subtract, Alu.mult)
                    if len(pend) >= 2:
                        bu_flush(pend)
                    pend.append((t, c0, w, G, sio))
                if n <= 2 * CH:
                    # small level: the next level's first chunk reads this
                    # level's last chunks -- deferral would skip the RAW dep
                    bu_flush(pend)

            # ---------------- top-down ----------------
            def td_phase2_one(st):
                t, coff, w, G, sio = st
                cdst = cbuf(t)[:, coff: coff + w]
                tch = W.tile([H, CH], BF16, tag="tc")
                nc.scalar.activation(out=tch[:, :w], in_=cdst,
                                     func=ActF.Tanh, scale=2.0)
                if t < DEPTH:
                    nc.vector.tensor_mul(hbuf(t)[:, coff: coff + w],
                                         sio[:, G:G + w], tch[:, :w])
                else:
                    # lone leaf chunk (only the final two, flushed singly
                    # to shorten the drain tail); odd chunks get their own
                    # slot group past the pair groups
                    assert w == CH
                    hn = W.tile([H, CH], BF16, tag="hn")
                    nc.vector.tensor_mul(hn[:, :w],
                                         sio[:, G:G + w], tch[:, :w])
                    ci = coff // CH
                    qoff = ci * 16 + (16 if ci % 2 == 1 else 0)
                    r2 = W.tile([H, CH], F32, tag="r2")
                    nc.vector.tensor_add(r2[:, 0:256], hn[:, 0:256],
                                         hn[:, 256:512])
                    nc.vector.tensor_add(r2[:, 256:384], r2[:, 0:128],
                                         r2[:, 128:256])
                    nc.vector.tensor_add(r2[:, 384:448], r2[:, 256:320],
                                         r2[:, 320:384])
                    nc.vector.tensor_add(slots[:, qoff:qoff + 32],
                                         r2[:, 384:416], r2[:, 416:448])

            def td_phase2_pair(s1, s2):
                t, coff, w, G, sio1 = s1
                _, coffb, wb, Gb, sio2 = s2
                tch = WD.tile([H, 2 * CH], BF16, tag="tc2")
                nc.scalar.activation(out=tch[:, :w + wb],
                                     in_=cbuf(t)[:, coff: coff + w + wb],
                                     func=ActF.Tanh, scale=2.0)
                if t < DEPTH:
                    nc.vector.tensor_mul(hbuf(t)[:, coff: coff + w],
                                         sio1[:, G:G + w], tch[:, :w])
                    nc.vector.tensor_mul(hbuf(t)[:, coffb: coffb + wb],
                                         sio2[:, Gb:Gb + wb], tch[:, w:w + wb])
                else:
                    assert w == CH and wb == CH
                    hn = W.tile([H, 2 * CH], BF16, tag="hn2")
                    nc.vector.tensor_mul(hn[:, :w],
                                         sio1[:, G:G + w], tch[:, :w])
                    nc.vector.tensor_mul(hn[:, w:w + wb],
                                         sio2[:, Gb:Gb + wb], tch[:, w:w + wb])
                    ci = coff // CH
                    # tree ids align mod 32 across both chunks: per-tree
                    # sums via fp32 halving adds (cheaper than the strided
                    # 32-group reduce on this engine)
                    r1 = W.tile([H, CH], F32, tag="r1")
                    nc.vector.tensor_add(r1[:, 0:512], hn[:, 0:512],
                                         hn[:, 512:1024])
                    r2 = W.tile([H, CH], F32, tag="r2")
                    nc.vector.tensor_add(r2[:, 0:256], r1[:, 0:256],
                                         r1[:, 256:512])
                    nc.vector.tensor_add(r2[:, 256:384], r2[:, 0:128],
                                         r2[:, 128:256])
                    nc.vector.tensor_add(r2[:, 384:448], r2[:, 256:320],
                                         r2[:, 320:384])
                    nc.vector.tensor_add(slots[:, ci * 16:ci * 16 + 32],
                                         r2[:, 384:416], r2[:, 416:448])

            def td_flush(pend):
                while pend:
                    if (len(pend) >= 2 and pend[0][0] == pend[1][0]
                            and pend[1][1] == pend[0][1] + pend[0][2]):
                        td_phase2_pair(pend[0], pend[1])
                        del pend[:2]
                    else:
                        td_phase2_one(pend[0])
                        del pend[:1]

            pend = []
            for t in range(0, DEPTH + 1):
                n = NT[t]
                xoff = OFF[t]
                cw = cbuf(t)
                if t == 0:
                    w = n  # 32
                    G = w
                    iou = PI.tile([H, 3 * CH], F32, tag="iou")
                    for k in range(3):
                        nc.tensor.matmul(iou[:, k * G: k * G + w],
                                         wtdx[:, k * H:(k + 1) * H],
                                         xt[:, xoff: xoff + w],
                                         start=(k == 0), stop=False)
                        nc.tensor.matmul(iou[:, k * G: k * G + w],
                                         wtdh[:, k * H:(k + 1) * H],
                                         hbu[:, xoff: xoff + w],
                                         start=False, stop=True)
                    sio = WD.tile([H, 3 * CH], BF16, tag="sio")
                    nc.scalar.activation(out=sio[:, :3 * G],
                                         in_=iou[:, 0:3 * G],
                                         func=ActF.Sigmoid)
                    nc.vector.scalar_tensor_tensor(
                        cw[:, 0:w], sio[:, 2 * G:2 * G + w], 0.5,
                        sio[:, 0:w], Alu.subtract, Alu.mult)
                    td_phase2_one((0, 0, w, G, sio))
                    continue
                half = n // 2
                hp = hbuf(t - 1)
                cp = cbuf(t - 1)
                if half <= CH:
                    # chain level (one parent chunk): X-dependent matmuls
                    # first so the PE chews them while the parent h chain
                    # resolves; psf/utd land right when hp is ready
                    w = half
                    G = w if w < CH else CH
                    ious = []
                    for side in range(2):
                        coff = side * w
                        iou = PI.tile([H, 3 * CH], F32, tag="iou")
                        for k in range(3):
                            nc.tensor.matmul(
                                iou[:, k * G: k * G + w],
                                wtdx[:, k * H:(k + 1) * H],
                                xt[:, xoff + coff: xoff + coff + w],
                                start=(k * G) % 512 == 0, stop=False)
                        for k in range(3):
                            nc.tensor.matmul(
                                iou[:, k * G: k * G + w],
                                wtdh[:, k * H:(k + 1) * H],
                                hbu[:, xoff + coff: xoff + coff + w],
                                start=False, stop=False)
                        ious.append(iou)
                    psf = PF.tile([H, 2 * CH], F32, tag="pf")
                    nc.tensor.matmul(psf[:, 0:w], uftd, hp[:, 0:w],
                                     start=True, stop=True)
                    for side in range(2):
                        for k in range(3):
                            nc.tensor.matmul(
                                ious[side][:, k * G: k * G + w],
                                utd[:, k * H:(k + 1) * H],
                                hp[:, 0:w], start=False, stop=True)
                    f = WD.tile([H, 2 * CH], BF16, tag="f")
                    nc.scalar.activation(out=f[:, :w], in_=psf[:, :w],
                                         func=ActF.Sigmoid, bias=bftd)
                    sios = []
                    for side in range(2):
                        sio = WD.tile([H, 3 * CH], BF16, tag="sio")
                        nc.scalar.activation(out=sio[:, :3 * G],
                                             in_=ious[side][:, 0:3 * G],
                                             func=ActF.Sigmoid)
                        sios.append(sio)
                    fc = W.tile([H, 2 * CH], BF16, tag="fcl")
                    nc.vector.tensor_mul(fc[:, :w], f[:, :w], cp[:, 0:w])
                    for side in range(2):
                        coff = side * w
                        t1 = W.tile([H, CH], BF16, tag="t1")
                        nc.vector.scalar_tensor_tensor(
                            t1[:, :w], sios[side][:, 2 * G:2 * G + w], 0.5,
                            sios[side][:, 0:w], Alu.subtract, Alu.mult)
                        nc.vector.tensor_add(cw[:, coff: coff + w],
                                             t1[:, :w], fc[:, :w])
                        pend.append((t, coff, w, G, sios[side]))
                    td_flush(pend)
                    continue
                # parent chunks processed in pairs sharing one psf tile and
                # one forget-gate sigmoid / fc multiply
                p0 = 0
                while p0 < half:
                    npair = 2 if p0 + CH < half else 1
                    wps = [min(CH, half - (p0 + j * CH)) for j in range(npair)]
                    psf = PF.tile([H, 2 * CH], F32, tag="pf")
                    for j in range(npair):
                        nc.tensor.matmul(psf[:, j * CH: j * CH + wps[j]],
                                         uftd,
                                         hp[:, p0 + j * CH: p0 + j * CH + wps[j]],
                                         start=True, stop=True)
                    fw = (npair - 1) * CH + wps[-1]
                    f = WD.tile([H, 2 * CH], BF16, tag="f")
                    nc.scalar.activation(out=f[:, :fw], in_=psf[:, :fw],
                                         func=ActF.Sigmoid, bias=bftd)
                    fc = W.tile([H, 2 * CH], BF16, tag="fcl")
                    nc.vector.tensor_mul(fc[:, :fw], f[:, :fw],
                                         cp[:, p0: p0 + fw])
                    for j in range(npair):
                        pj = p0 + j * CH
                        w = wps[j]
                        G = w if w < CH else CH
                        for side in range(2):
                            coff = 2 * pj + side * w
                            iou = PI.tile([H, 3 * CH], F32, tag="iou")
                            for k in range(3):
                                nc.tensor.matmul(
                                    iou[:, k * G: k * G + w],
                                    wtdx[:, k * H:(k + 1) * H],
                                    xt[:, xoff + coff: xoff + coff + w],
                                    start=(k * G) % 512 == 0,
                                    stop=False)
                            for k in range(3):
                                nc.tensor.matmul(
                                    iou[:, k * G: k * G + w],
                                    wtdh[:, k * H:(k + 1) * H],
                                    hbu[:, xoff + coff: xoff + coff + w],
                                    start=False, stop=False)
                            for k in range(3):
                                nc.tensor.matmul(
                                    iou[:, k * G: k * G + w],
                                    utd[:, k * H:(k + 1) * H],
                                    hp[:, pj: pj + w],
                                    start=False, stop=True)
                            sio = WD.tile([H, 3 * CH], BF16, tag="sio")
                            fcs = fc[:, j * CH: j * CH + w]
                            nc.scalar.activation(out=sio[:, :3 * G],
                                                 in_=iou[:, 0:3 * G],
                                                 func=ActF.Sigmoid)
                            t1 = W.tile([H, CH], BF16, tag="t1")
                            nc.vector.scalar_tensor_tensor(
                                t1[:, :w], sio[:, 2 * G:2 * G + w], 0.5,
                                sio[:, 0:w], Alu.subtract, Alu.mult)
                            nc.vector.tensor_add(
                                cw[:, coff: coff + w], t1[:, :w], fcs)
                            if len(pend) >= 2:
                                td_flush(pend)
                            pend.append((t, coff, w, G, sio))
                    p0 += npair * CH
                if n <= 2 * CH:
                    td_flush(pend)
            # flush the final leaf chunks SINGLY: the second-to-last
            # chunk's reduction then overlaps the last chunk's gate math,
            # shortening the serial drain tail
            while pend:
                td_phase2_one(pend.pop(0))

            # leaf mean: sum the 9 partial groups, scale by 1/256
            nc.vector.reduce_sum(
                out=outsb[:, 32:64],
                in_=slots[:, 0:288].rearrange("p (k t) -> p t k", t=32),
                axis=mybir.AxisListType.X)
            nc.vector.tensor_scalar_mul(outsb[:, 32:64], outsb[:, 32:64],
                                        1.0 / 256.0)
            nc.sync.dma_start(out=OUT[:], in_=outsb[:])

    _split_multi_waits(nc)
    return nc


def _perm():
    """Per-core node permutation: level-grouped chunk-local-split order.
    Entry = row index into the core's [16352, 128] X slab."""
    trees = np.arange(TPC, dtype=np.int64)
    heap = [np.zeros(TPC, dtype=np.int64)]
    tree = [trees.copy()]
    for t in range(1, DEPTH + 1):
        ph, pt = heap[t - 1], tree[t - 1]
        nh, ntr = [], []
        for j in range(0, len(ph), CH):
            bh = ph[j:j + CH]
            bt = pt[j:j + CH]
            nh.append(2 * bh + 1)
            nh.append(2 * bh + 2)
            ntr.append(bt)
            ntr.append(bt)
        heap.append(np.concatenate(nh))
        tree.append(np.concatenate(ntr))
    parts = [tree[t] * M + heap[t] for t in range(DEPTH, -1, -1)]
    return np.concatenate(parts)


def kernel(**inputs):
    from concourse.bass_utils import run_bass_kernel_spmd

    X = np.asarray(inputs["X"], dtype=np.float32)
    W_iou_bu = np.asarray(inputs["W_iou_bu"], dtype=np.float32)
    U_iou_bu = np.asarray(inputs["U_iou_bu"], dtype=np.float32)
    Uf_bu = np.asarray(inputs["Uf_bu"], dtype=np.float32)
    bf_bu = np.asarray(inputs["bf_bu"], dtype=np.float32)
    W_iou_td = np.asarray(inputs["W_iou_td"], dtype=np.float32)
    U_iou_td = np.asarray(inputs["U_iou_td"], dtype=np.float32)
    Uf_td = np.asarray(inputs["Uf_td"], dtype=np.float32)
    bf_td = np.asarray(inputs["bf_td"], dtype=np.float32)

    bf16 = ml_dtypes.bfloat16
    wpack = np.concatenate([
        W_iou_bu.T, U_iou_bu.T, Uf_bu.T,
        W_iou_td[:, :H].T, W_iou_td[:, H:].T, U_iou_td.T, Uf_td.T,
    ], axis=1).copy()
    # u-gate rows pre-scaled by 2: tanh(u) = 2*sigmoid(2u) - 1 on device
    for base in (0, 384, 896, 1280, 1664):
        wpack[:, base + 2 * H: base + 3 * H] *= 2.0
    bpack = np.stack([bf_bu, bf_td], axis=1)
    shared = {
        "WPACK": np.ascontiguousarray(wpack).astype(bf16),
        "BPACK": np.ascontiguousarray(bpack, dtype=np.float32),
    }
    perm = _perm()
    in_maps = []
    for c in range(NCORES):
        slab = X[c * NC_NODES:(c + 1) * NC_NODES]
        xtc = np.ascontiguousarray(slab[perm].T.astype(bf16))
        m = dict(shared)
        m["XT"] = xtc
        in_maps.append(m)

    if "nc" not in _NC_CACHE:
        _NC_CACHE["nc"] = _build_nc()
    nc = _NC_CACHE["nc"]

    trace = bool(os.environ.get("BIDI_TRACE"))
    if trace:
        sys.path.insert(0, "/root/problem/work")
        try:
            import ntff_hook
            ntff_hook.install()
        except Exception:
            trace = False
    res = run_bass_kernel_spmd(nc, in_maps, core_ids=list(range(NCORES)),
                               trace=trace)
    global LAST_EXEC_NS, LAST_TRACE
    LAST_EXEC_NS = res.exec_time_ns
    LAST_TRACE = res.instructions_and_trace

    out = np.empty((B, 2 * H), dtype=np.float32)
    for c in range(NCORES):
        o = res.results[c]["OUT"]          # [128, 64]
        out[c * TPC:(c + 1) * TPC, :H] = o[:, 0:32].T
        out[c * TPC:(c + 1) * TPC, H:] = o[:, 32:64].T
    return out


LAST_EXEC_NS = None
LAST_TRACE = None
